# revision 75
# baseline (speedup 1.0000x reference)
"""TRN2 Bass kernel for nn_Attention_49778670961018 (gnn_message_passing).

Math (per reference):
    S_ss = (Xs @ W_ss.T + b_ss) @ A.T ; S_es = (Xe @ W_es.T + b_es) @ A.T
    w_*  = softmax(S_*, axis=0)   (biases shift columns uniformly -> dropped)
    ctx_ss = w_ss.T @ Xs ; ctx_es = w_es.T @ Xe
    out  = tanh([A | ctx_ss | ctx_es] @ W_lin.T + b_lin)

Sharding: attender rows (M=8192) split across 8 cores (1024 each).

Per core, zero PE transposes; all matmuls 16-bit at full PE rate:
    score path in fp16 (11-bit mantissa keeps softmax-sensitive score
    noise ~8x below bf16), aggregation/final in bf16 (exp output needs
    bf16's exponent range: values reach ~e^-120).
    P0  cast A/X to fp16 DRAM (gpsimd), A^T via DMA-xbar transpose;
        Q^T = W^T A^T on PE; stability bound c[m] = coef*||q_m|| + 40
        from Q norms (ones-matmul trick).
    P1  S^T = Q X^T with X^T tiles DMA-transposed from the fp16 X copy;
        exp on ACT with per-partition bias, Z via accum_out; E^T -> DRAM
        as a 2D [M_loc, N] bf16 matrix.
    P2  ctx^T[h,m] = sum_n X[n,h] E[n,m]: X natural tiles (fp16 load +
        DVE convert to bf16) stationary, E natural (DMA-transposed back)
        moving; UNNORMALIZED ctx^T stays resident in SBUF - no ctx DRAM
        round trip.
    P3/P4 same for the es attendee set.
    P5  final linear with three split PSUM accumulators (A-part+bias,
        ss-part, es-part); 1/Z normalization applied as per-partition
        scalars during the DVE combine, then tanh on ACT.
"""
import os
import sys

import numpy as np

sys.path.insert(0, "/opt/trn_rl_repo")

import concourse.bass as bass  # noqa: E402
import concourse.mybir as mybir  # noqa: E402
import concourse.tile as tile  # noqa: E402
from concourse import bacc  # noqa: E402
from concourse.bass_utils import run_bass_kernel_spmd  # noqa: E402
from concourse.masks import make_identity  # noqa: E402

F32 = mybir.dt.float32
F16 = mybir.dt.float16
BF16 = mybir.dt.bfloat16
AX = mybir.AxisListType
AF = mybir.ActivationFunctionType
ALU = mybir.AluOpType

H = 1024          # hidden dim
HS = H // 128     # h-slices
NCORES = 8
MLOC = 1024       # attender rows per core
MT = MLOC // 128  # m-tiles per core
NCH = 512         # attendee chunk (score matmul free dim)
CAST_ROWS = 2048  # X fp16-cast chunk
CMAX_MARGIN = 40.0


def _max_coef(n):
    """E[max of n iid N(0,1)] (Gumbel asymptotic)."""
    a = np.sqrt(2 * np.log(n))
    return float(a - (np.log(np.log(n)) + np.log(4 * np.pi)) / (2 * a))


def _cast_x_chunks(nc, x_dram, x_f16s):
    """DRAM->DRAM fp32->fp16 cast, one CAST_ROWS-row tensor per chunk."""
    for c, t in enumerate(x_f16s):
        nc.gpsimd.dma_start(t.ap()[:],
                            x_dram[c * CAST_ROWS:(c + 1) * CAST_ROWS, :])


def _xslice(x_f16s, r0, nrow):
    """AP for rows [r0, r0+nrow) of the chunked fp16 X copy."""
    c = r0 // CAST_ROWS
    o = r0 % CAST_ROWS
    assert o + nrow <= CAST_ROWS
    return x_f16s[c].ap()[o:o + nrow, :]


def _scores_phase(nc, tc, sfx, x_f16s, n_rows, qt, cneg, zcols, etT_ps,
                  xtp, ebfp, after_pair0=None):
    """S^T = Q X^T per 1024-col pair; exp -> E^T [MLOC, 1024] bf16 per pair."""
    npair = n_rows // 1024
    with (
        tc.tile_pool(name=f"sps{sfx}", bufs=4, space="PSUM") as sps,
    ):
        for p in range(npair):
            xts = []
            for half in range(2):
                xt = xtp.tile([128, HS, NCH], F16, tag="xt", name="xt")
                r0 = (2 * p + half) * NCH
                nc.sync.dma_start_transpose(xt[:], _xslice(x_f16s, r0, NCH))
                xts.append(xt)
            for mt in range(MT):
                sp = sps.tile([128, 2 * NCH], F32, tag="sp", name=f"sp{sfx}")
                for half in range(2):
                    dst = sp[:, half * NCH:(half + 1) * NCH]
                    for h in range(HS):
                        nc.tensor.matmul(dst, qt[:, h, mt * 128:(mt + 1) * 128],
                                         xts[half][:, h, :],
                                         start=(h == 0), stop=(h == HS - 1))
                e_bf = ebfp.tile([128, 2 * NCH], BF16, tag="ebf", name="eb")
                nc.scalar.activation(e_bf[:], sp[:], AF.Exp,
                                     bias=cneg[:, mt:mt + 1],
                                     accum_out=zcols[:, mt, p:p + 1])
                nc.sync.dma_start(
                    etT_ps[p].ap()[mt * 128:(mt + 1) * 128, :], e_bf[:])


def _agg_prefetch(nc, x_f16s, etT_ps, agg0):
    """Load both mh-passes' g=0 tiles into the long-lived agg0 pool.

    Emitted BEFORE the producing scores phase ends: the deps are only
    pair-0's E^T writes and the fp16 X cast, both ready early, and the
    fresh pool slots carry no WAR against the running phase's tiles.
    """
    pre = {}
    for mh in range(2):
        en = agg0.tile([128, 8, 512], BF16, tag="en0", name="en0")
        nc.scalar.dma_start_transpose(
            en[:], etT_ps[0].ap()[mh * 512:(mh + 1) * 512, :])
        xns = []
        if mh == 0:
            for t in range(2):
                xf = agg0.tile([128, 4, H], F16, tag="xf0", name="xf0")
                nc.gpsimd.dma_start(
                    xf[:], _xslice(x_f16s, t * 512, 512).rearrange(
                        "(a p) h -> p a h", p=128))
                xn = agg0.tile([128, 4, H], BF16, tag="xn0", name="xn0")
                nc.vector.tensor_copy(xn[:], xf[:])
                xns.append(xn)
        pre[mh] = (en, xns)
    return pre


def _agg_phase(nc, tc, sfx, x_f16s, n_rows, etT_ps, ctxT, pre=None,
               xf_eng0=None):
    """ctx^T[h,m] = sum_n X[n,h] E[n,m], unnormalized, resident bf16.

    X natural tiles stream per mh-pass: fp16 plain load + DVE convert to
    bf16 (dtype must match E). E is read back transposed from the
    per-pair E^T tensors -> [128, 8, 512] (n-slice, m-half) tiles.
    """
    NG = n_rows // 1024         # groups of 8 n-slices
    with (
        tc.tile_pool(name=f"xf{sfx}", bufs=3) as xfp,
        tc.tile_pool(name=f"xn{sfx}", bufs=4) as xnp,
        tc.tile_pool(name=f"en{sfx}", bufs=4) as enp,
        tc.tile_pool(name=f"cps{sfx}", bufs=8, space="PSUM") as cps,
    ):
        for mh in range(2):
            ctx_ps = [cps.tile([128, 512], F32, tag="cp", name=f"cp{sfx}")
                      for _ in range(HS)]
            for g in range(NG):
                en = enp.tile([128, 8, 512], BF16, tag="en", name=f"en{sfx}")
                nc.sync.dma_start_transpose(
                    en[:], etT_ps[g].ap()[mh * 512:(mh + 1) * 512, :])
                xns = []
                for t in range(2):
                    xf = xfp.tile([128, 4, H], F16, tag="xf", name=f"xf{sfx}")
                    r0 = (2 * g + t) * 512
                    eng = (xf_eng0 or nc.gpsimd) if t == 0 else nc.scalar
                    eng.dma_start(
                        xf[:], _xslice(x_f16s, r0, 512).rearrange(
                            "(a p) h -> p a h", p=128))
                    xn = xnp.tile([128, 4, H], BF16, tag="xn", name=f"xn{sfx}")
                    nc.vector.tensor_copy(xn[:], xf[:])
                    xns.append(xn)
                if g < NG - 1:
                    for jj in range(8):
                        xn = xns[jj // 4]
                        for h in range(HS):
                            nc.tensor.matmul(
                                ctx_ps[h][:],
                                xn[:, jj % 4, h * 128:(h + 1) * 128],
                                en[:, jj, :],
                                start=(g == 0 and jj == 0), stop=False)
                else:
                    # last group h-outer so each h finishes (and evacuates)
                    # early, releasing its PSUM bank before the phase ends
                    for h in range(HS):
                        for jj in range(8):
                            xn = xns[jj // 4]
                            nc.tensor.matmul(
                                ctx_ps[h][:],
                                xn[:, jj % 4, h * 128:(h + 1) * 128],
                                en[:, jj, :],
                                start=(g == 0 and jj == 0), stop=(jj == 7))
                        nc.vector.tensor_copy(
                            ctxT[:, h, mh * 512:(mh + 1) * 512], ctx_ps[h][:])


def build():
    NS_ROWS = int(os.environ.get("KNS", 8192))
    NE_ROWS = int(os.environ.get("KNE", 4096))
    krepeat = int(os.environ.get("KREPEAT", "1"))
    nc = bacc.Bacc("TRN2", target_bir_lowering=False, debug=False,
                   num_devices=NCORES)

    xs = nc.dram_tensor("attendee_stmts", [NS_ROWS, H], F32, kind="ExternalInput").ap()
    xe = nc.dram_tensor("attendee_eres", [NE_ROWS, H], F32, kind="ExternalInput").ap()
    al = nc.dram_tensor("attender_loc", [MLOC, H], F32, kind="ExternalInput").ap()
    wss = nc.dram_tensor("W_ss", [H, H], F32, kind="ExternalInput").ap()
    wes = nc.dram_tensor("W_es", [H, H], F32, kind="ExternalInput").ap()
    wlin = nc.dram_tensor("W_lin", [H, 3 * H], F32, kind="ExternalInput").ap()
    blin = nc.dram_tensor("b_lin", [H], F32, kind="ExternalInput").ap()
    out = nc.dram_tensor("out", [MLOC, H], F32, kind="ExternalOutput").ap()

    # DRAM scratch, chunked into separate tensors so the dependency
    # tracker sees no false WAR/RAW between chunk writers and readers
    NP_S, NP_E = NS_ROWS // 1024, NE_ROWS // 1024
    xs_f16s = [nc.dram_tensor(f"xs_f16_{c}", [CAST_ROWS, H], F16)
               for c in range(NS_ROWS // CAST_ROWS)]
    xe_f16s = [nc.dram_tensor(f"xe_f16_{c}", [CAST_ROWS, H], F16)
               for c in range(NE_ROWS // CAST_ROWS)]
    wlin_bf = nc.dram_tensor("wlin_bf", [H, 3 * H], BF16)
    essT_ps = [nc.dram_tensor(f"essT_{p}", [MLOC, 1024], BF16)
               for p in range(NP_S)]
    eesT_ps = [nc.dram_tensor(f"eesT_{p}", [MLOC, 1024], BF16)
               for p in range(NP_E)]

    with tile.TileContext(nc) as tc:
      for rep in range(krepeat):
        R = f"r{rep}" if rep else ""
        with tc.tile_pool(name=f"small{R}", bufs=1) as small:
            cneg_ss = small.tile([128, MT], F32)
            cneg_es = small.tile([128, MT], F32)
            zc_ss = small.tile([128, MT, NP_S], F32)
            zc_es = small.tile([128, MT, NP_E], F32)
            rz_ss = small.tile([128, MT], F32)
            rz_es = small.tile([128, MT], F32)
            nc.vector.memset(zc_ss[:], 0.0)
            nc.vector.memset(zc_es[:], 0.0)

            # ---------------- P0: casts, A^T, Q^T, c[m] ----------------
            with tc.tile_pool(name=f"ctxs{R}", bufs=1) as ctxsp:
             with (
                 tc.tile_pool(name=f"qte{R}", bufs=1) as qtep,
                 tc.tile_pool(name=f"xtsh{R}", bufs=4) as xtp,
                 tc.tile_pool(name=f"ebsh{R}", bufs=4) as ebfp,
             ):
              with tc.tile_pool(name=f"qts{R}", bufs=1) as qtsp:
                qt_ss = qtsp.tile([128, HS, MLOC], F16, name="qt_ss")
                qt_es = qtep.tile([128, HS, MLOC], F16, name="qt_es")
                with (
                    tc.tile_pool(name=f"p0at{R}", bufs=1) as p0at,
                    tc.tile_pool(name=f"p0w{R}", bufs=2) as p0w,
                    tc.tile_pool(name=f"p0{R}", bufs=2) as p0,
                    tc.tile_pool(name=f"p0a{R}", bufs=2) as p0a,
                    tc.tile_pool(name=f"p0s{R}", bufs=1) as p0s,
                    tc.tile_pool(name=f"p0ps{R}", bufs=3, space="PSUM") as p0ps,
                    tc.tile_pool(name=f"qnps{R}", bufs=2, space="PSUM") as qnps,
                    tc.tile_pool(name=f"atps{R}", bufs=2, space="PSUM") as atps,
                ):
                    # A^T via PE transposes of fp32 A tiles (baseline-proven
                    # pattern); PE warms from ~2us, no DRAM cast on the
                    # critical path
                    at_sb = p0at.tile([128, HS, MLOC], F16, name="at_sb")
                    ident = p0s.tile([128, 128], F32, name="ident")
                    make_identity(nc, ident[:])
                    ones_f16 = p0s.tile([128, 1], F16, name="ones_f16")
                    nc.vector.memset(ones_f16[:], 1.0)

                    # both W loads up front (gpsimd), then X casts behind them
                    w_sbs = []
                    for w_dram in (wss, wes):
                        w_sb = p0w.tile([128, HS, H], F16, tag="w", name="w_sb")
                        nc.gpsimd.dma_start(
                            w_sb[:], w_dram.rearrange("(a p) j -> p a j", p=128))
                        w_sbs.append(w_sb)
                    _cast_x_chunks(nc, xs, xs_f16s)
                    _cast_x_chunks(nc, xe, xe_f16s)
                    nc.gpsimd.dma_start(wlin_bf.ap()[:], wlin)

                    for mt in range(MT):
                        a_t = p0a.tile([128, H], F32, tag="ald", name="a_t")
                        nc.scalar.dma_start(
                            a_t[:], al[mt * 128:(mt + 1) * 128, :])
                        for hp in range(HS // 2):
                            pt = atps.tile([128, 256], F32, tag="pt",
                                           name="pt")
                            for i in range(2):
                                h = hp * 2 + i
                                nc.tensor.transpose(
                                    pt[:, i * 128:(i + 1) * 128],
                                    a_t[:, h * 128:(h + 1) * 128], ident[:])
                            for i in range(2):
                                h = hp * 2 + i
                                nc.vector.tensor_copy(
                                    at_sb[:, h, mt * 128:(mt + 1) * 128],
                                    pt[:, i * 128:(i + 1) * 128])

                    for wi, (w_sb, qt, coef, cneg) in enumerate([
                            (w_sbs[0], qt_ss, _max_coef(NS_ROWS), cneg_ss),
                            (w_sbs[1], qt_es, _max_coef(NE_ROWS), cneg_es)]):
                        # ||q_m||^2 accumulated directly in [m-part, mt]
                        # layout: per m-tile, lhsT = squared Q^T slice,
                        # rhs = ones column -> [128, 1] psum column
                        qn_ps = qnps.tile([128, MT], F32, tag="qn",
                                          name="qn_ps")
                        for hj in range(HS):
                            for mh in range(2):
                                qp = p0ps.tile([128, 512], F32, tag="qp",
                                               name="qp")
                                for a in range(HS):
                                    nc.tensor.matmul(
                                        qp[:], w_sb[:, a, hj * 128:(hj + 1) * 128],
                                        at_sb[:, a, mh * 512:(mh + 1) * 512],
                                        start=(a == 0), stop=(a == HS - 1))
                                nc.vector.tensor_copy(
                                    qt[:, hj, mh * 512:(mh + 1) * 512], qp[:])
                                qsq = p0.tile([128, 512], F16, tag="qsq",
                                              name="qsq")
                                nc.scalar.activation(qsq[:], qp[:], AF.Square)
                                for ml in range(4):
                                    mt = mh * 4 + ml
                                    nc.tensor.matmul(
                                        qn_ps[:, mt:mt + 1],
                                        qsq[:, ml * 128:(ml + 1) * 128],
                                        ones_f16[:], start=(hj == 0),
                                        stop=(hj == HS - 1),
                                        skip_group_check=True)
                        qn_sb = p0.tile([128, MT], F32, tag="qn_sb",
                                        name="qn_sb")
                        nc.scalar.activation(qn_sb[:], qn_ps[:], AF.Sqrt)
                        nc.vector.tensor_scalar(cneg[:], qn_sb[:], -coef,
                                                -CMAX_MARGIN, op0=ALU.mult,
                                                op1=ALU.add)

                # bf16 A^T for the final linear: DVE is idle here and
                # at_sb is complete once the Q matmuls are emitted
                at2 = ctxsp.tile([128, HS, MLOC], BF16, name="at2")
                nc.vector.tensor_copy(at2[:], at_sb[:])

                # ---------------- P1: ss scores ----------------
                _scores_phase(nc, tc, f"s{R}", xs_f16s, NS_ROWS, qt_ss,
                              cneg_ss, zc_ss, essT_ps, xtp, ebfp)

              # ---------------- P2: ss aggregation ----------------
              for mt in range(MT):
                  nc.vector.tensor_reduce(rz_ss[:, mt:mt + 1],
                                          zc_ss[:, mt, :], axis=AX.X,
                                          op=ALU.add)
              nc.vector.reciprocal(rz_ss[:], rz_ss[:])
              ctxT_ss = ctxsp.tile([128, HS, MLOC], BF16, name="ctxT_ss")
              _agg_phase(nc, tc, f"s{R}", xs_f16s, NS_ROWS, essT_ps, ctxT_ss)

              # ---------------- P3: es scores ----------------
              _scores_phase(nc, tc, f"e{R}", xe_f16s, NE_ROWS, qt_es,
                            cneg_es, zc_es, eesT_ps, xtp, ebfp)

             # ---------------- P4: es aggregation ----------------
             if True:
                    for mt in range(MT):
                        nc.vector.tensor_reduce(rz_es[:, mt:mt + 1],
                                                zc_es[:, mt, :], axis=AX.X,
                                                op=ALU.add)
                    nc.vector.reciprocal(rz_es[:], rz_es[:])
                    with tc.tile_pool(name=f"p45{R}", bufs=1) as p45:
                        wlt = p45.tile([128, 3 * HS, H], BF16, name="wlt")
                        nc.sync.dma_start_transpose(wlt[:], wlin_bf.ap()[:])
                        blin_bf = p45.tile([1, H], BF16, name="blin_bf")
                        nc.gpsimd.dma_start(
                            blin_bf[:], blin.rearrange("(a h) -> a h", a=1))
                        ones_row = p45.tile([1, 128], BF16, name="ones_row")
                        nc.vector.memset(ones_row[:], 1.0)

                        ctxT_es = p45.tile([128, HS, MLOC], BF16, name="ctxT_es")
                        _agg_phase(nc, tc, f"e{R}", xe_f16s, NE_ROWS,
                                   eesT_ps, ctxT_es, xf_eng0=nc.sync)

                        # ------------ P5: final linear + tanh ------------
                        with (
                            tc.tile_pool(name=f"p5o{R}", bufs=3) as p5o,
                            tc.tile_pool(name=f"p5a{R}", bufs=2,
                                         space="PSUM") as p5a,
                            tc.tile_pool(name=f"p5b{R}", bufs=1,
                                         space="PSUM") as p5b,
                        ):
                            for mt in range(MT):
                                msl = slice(mt * 128, (mt + 1) * 128)
                                for ah in range(2):
                                    asl = slice(ah * 512, (ah + 1) * 512)
                                    b0 = p5a.tile([128, 512], F32, tag="b0",
                                                  name="b0")
                                    b1 = p5b.tile([128, 512], F32, tag="b1",
                                                  name="b1")
                                    b2 = p5b.tile([128, 512], F32, tag="b2",
                                                  name="b2")
                                    nc.tensor.matmul(b0[:], ones_row[0:1, :],
                                                     blin_bf[0:1, asl],
                                                     start=True, stop=False)
                                    for s in range(HS):
                                        nc.tensor.matmul(
                                            b0[:], at2[:, s, msl],
                                            wlt[:, s, asl],
                                            start=False, stop=(s == HS - 1))
                                    for s in range(HS):
                                        nc.tensor.matmul(
                                            b1[:], ctxT_ss[:, s, msl],
                                            wlt[:, HS + s, asl],
                                            start=(s == 0), stop=(s == HS - 1))
                                    for s in range(HS):
                                        nc.tensor.matmul(
                                            b2[:], ctxT_es[:, s, msl],
                                            wlt[:, 2 * HS + s, asl],
                                            start=(s == 0), stop=(s == HS - 1))
                                    t1 = p5o.tile([128, 512], F32, tag="t1",
                                                  name="t1")
                                    nc.vector.tensor_scalar_mul(
                                        t1[:], b1[:], rz_ss[:, mt:mt + 1])
                                    t2 = p5o.tile([128, 512], F32, tag="t2",
                                                  name="t2")
                                    nc.vector.tensor_scalar_mul(
                                        t2[:], b2[:], rz_es[:, mt:mt + 1])
                                    nc.vector.tensor_tensor(
                                        t1[:], t1[:], t2[:], op=ALU.add)
                                    nc.vector.tensor_tensor(
                                        t1[:], t1[:], b0[:], op=ALU.add)
                                    o_sb = p5o.tile([128, 512], F32, tag="o",
                                                    name="o_sb")
                                    nc.scalar.activation(o_sb[:], t1[:],
                                                         AF.Tanh)
                                    nc.sync.dma_start(out[msl, asl], o_sb[:])

    nc.compile()
    return nc


_NC_CACHE = None


def kernel(**inputs):
    global _NC_CACHE
    xs = np.ascontiguousarray(np.asarray(inputs["attendee_stmts"], dtype=np.float32))
    xe = np.ascontiguousarray(np.asarray(inputs["attendee_eres"], dtype=np.float32))
    att = np.ascontiguousarray(np.asarray(inputs["attender"], dtype=np.float32))
    wss = np.ascontiguousarray(np.asarray(inputs["W_ss"], dtype=np.float32))
    wes = np.ascontiguousarray(np.asarray(inputs["W_es"], dtype=np.float32))
    wlin = np.ascontiguousarray(np.asarray(inputs["W_lin"], dtype=np.float32))
    blin = np.ascontiguousarray(np.asarray(inputs["b_lin"], dtype=np.float32))

    if _NC_CACHE is None:
        _NC_CACHE = build()
    nc = _NC_CACHE

    in_maps = []
    for c in range(NCORES):
        in_maps.append({
            "attendee_stmts": xs,
            "attendee_eres": xe,
            "attender_loc": np.ascontiguousarray(att[c * MLOC:(c + 1) * MLOC, :]),
            "W_ss": wss,
            "W_es": wes,
            "W_lin": wlin,
            "b_lin": blin,
        })
    trace = bool(int(os.environ.get("KTRACE", "0")))
    res = run_bass_kernel_spmd(nc, in_maps, core_ids=list(range(NCORES)),
                               trace=trace)
    global LAST_RESULTS
    LAST_RESULTS = res
    return np.concatenate(
        [res.results[c]["out"] for c in range(NCORES)], axis=0).astype(np.float32)


LAST_RESULTS = None
